# revision 4
# baseline (speedup 1.0000x reference)
"""Trainium2 Bass kernel: 3-level db4 DWT front-end via PE banded matmuls.

Input  x : [16, 128, 8192] float32
Output   : [4, 16, 128, 8192] float32  (bands: approx, d3, d2, d1)

Strategy: 2048 independent rows sharded 256/core (2 tiles of 128 rows).
Per 128-row tile, all compute runs on the TensorEngine in a transposed
"segment" layout (signal positions on partitions, rows on the free axis):

  1. transpose-in: 64 matmuls  xseg_b = xrow_b^T @ I   (fp16)
  2. analysis: each level's dec_lo/dec_hi stride-2 conv (with reflect pad
     + odd-length handling folded into per-block constants) is a banded
     linear map; out block s = sum_j Msub_j @ in block (2s+d_j). Emitted as
     const-stationary matmuls: lhsT = Msub^T (shared across a run of up to
     4 output blocks), rhs = stride-2 block view of the input seg tile.
  3. synthesis: each band's full conv_transpose chain (1-3 stages, crop 7
     each) is composed host-side into one banded map from the deepest
     intermediate straight to the 8192-length band. Emitted data-stationary:
     lhsT = input seg block (data), rhs = per-output-block constant
     (Gsub^T), so the output lands directly in row layout [rows, positions]
     in PSUM - no transpose-out needed.

All constants are extracted at import time by applying the (numpy replica
of the) reference pipeline to identity matrices, so reflect padding, crop
offsets and tail truncation are exact by construction. fp16 data/weights
with fp32 PSUM accumulation gives rel err ~4e-4 (tolerance 2e-2).

PSUM rules honored: one matmul never crosses a 2KB bank; per bank the
first matmul carries start=True (lazy-zeroes the bank), the last stop=True.
"""

import numpy as np

import concourse.bass as bass
import concourse.tile as tile
from concourse import bacc, mybir
from concourse.bass_utils import run_bass_kernel_spmd

F32 = mybir.dt.float32
F16 = mybir.dt.float16

K = 8
DEC_LO = np.array([-0.0105974018, 0.0328830117, 0.0308413818, -0.1870348117,
                   -0.0279837694, 0.6308807679, 0.7148465706, 0.2303778133], np.float64)
DEC_HI = np.array([-0.2303778133, 0.7148465706, -0.6308807679, -0.0279837694,
                   0.1870348117, 0.0308413818, -0.0328830117, -0.0105974018], np.float64)
REC_LO = DEC_LO[::-1].copy()
REC_HI = DEC_HI[::-1].copy()

L0, L1, L2, L3 = 8192, 4100, 2054, 1031
N_CORES = 8
ROWS_PER_CORE = 256
TILES_PER_CORE = 2
NB0 = L0 // 128          # 64 x-blocks
NB = {"xseg": NB0, "d1": 33, "a1": 33, "d2": 17, "a2": 17, "d3": 9, "a3": 9}


# ---------------------------------------------------------------------------
# numpy replica of the reference pipeline (for map extraction)
# ---------------------------------------------------------------------------

def _dwconv_stride2(x, w):
    xp = np.pad(x, ((0, 0), (K - 1, K - 1)), mode="reflect")
    No = (xp.shape[1] - K) // 2 + 1
    out = np.zeros((x.shape[0], No), x.dtype)
    for k in range(K):
        out += w[k] * xp[:, k:k + 2 * No - 1:2]
    return out


def _dwconvT_stride2(x, w):
    N, L = x.shape
    up = np.zeros((N, 2 * L - 1), x.dtype)
    up[:, ::2] = x
    wf = w[::-1]
    upp = np.pad(up, ((0, 0), (K - 1, K - 1)))
    out = np.zeros((N, 2 * L + K - 2), x.dtype)
    for k in range(K):
        out += wf[k] * upp[:, k:k + out.shape[1]]
    return out


def _fit(out, target_len):
    Lc = out.shape[-1]
    if Lc > target_len:
        s = (Lc - target_len) // 2
        return out[..., s:s + target_len]
    if Lc < target_len:
        return np.pad(out, ((0, 0), (0, target_len - Lc)))
    return out


def _extract(op, in_len, out_pad):
    """Map matrix [out_pad, in_len] of linear `op` (zero-padded rows)."""
    M = op(np.eye(in_len)).T
    Mp = np.zeros((out_pad, in_len), np.float64)
    Mp[:M.shape[0]] = M
    return Mp


def _build_maps():
    P1, P2, P3 = NB["d1"] * 128, NB["d2"] * 128, NB["d3"] * 128
    mp = {}
    for nm, w_, li, op_ in (("d1", DEC_HI, L0, P1), ("a1", DEC_LO, L0, P1),
                            ("d2", DEC_HI, L1, P2), ("a2", DEC_LO, L1, P2),
                            ("d3", DEC_HI, L2, P3), ("a3", DEC_LO, L2, P3)):
        M = _extract(lambda I, w=w_: _dwconv_stride2(I, w), li, op_)
        # zero-pad input length to whole blocks (padded tail cols are zero)
        Mp = np.zeros((op_, -(-li // 128) * 128), np.float64)
        Mp[:, :li] = M
        mp[nm] = Mp

    def synth(chain, targets):
        def op(I):
            rec = I
            for w_, tgt in zip(chain, targets):
                rec = _fit(_dwconvT_stride2(rec, w_), tgt)
            return rec
        return op

    for nm, ch, tg, li in (("b3", [REC_HI], [L0], L1),
                           ("b2", [REC_HI, REC_LO], [L1, L0], L2),
                           ("b1", [REC_HI, REC_LO, REC_LO], [L2, L1, L0], L3),
                           ("b0", [REC_LO, REC_LO, REC_LO], [L2, L1, L0], L3)):
        M = _extract(synth(ch, tg), li, L0)
        Mp = np.zeros((L0, -(-li // 128) * 128), np.float64)
        Mp[:, :li] = M
        mp[nm] = Mp
    return mp


def _blockize(M):
    ob, ib = M.shape[0] // 128, M.shape[1] // 128
    out = []
    for s in range(ob):
        row = M[s * 128:(s + 1) * 128]
        ents = []
        for bi in range(ib):
            sub = row[:, bi * 128:(bi + 1) * 128]
            if np.abs(sub).max() > 0:
                ents.append((bi, sub))
        out.append(ents)
    return out


# ---------------------------------------------------------------------------
# plan construction
# ---------------------------------------------------------------------------

class _Slots:
    """fp16 constant [128,128] slot registry; contiguous runs dedup'd whole,
    including containment in previously stored slot sequences."""

    def __init__(self):
        self.mats = []          # list of np [128,128] float16 (stored as M^T)
        self._keys = []         # per-slot content hash
        self._runs = {}         # key(tuple of hashes) -> slot0

    def alloc_run(self, mats):
        key = tuple(np.asarray(m.T, np.float16).tobytes() for m in mats)
        if key in self._runs:
            return self._runs[key]
        n = len(key)
        for s0 in range(len(self._keys) - n + 1):
            if tuple(self._keys[s0:s0 + n]) == key:
                self._runs[key] = s0
                return s0
        s0 = len(self.mats)
        for m, k in zip(mats, key):
            self.mats.append(np.asarray(m.T, np.float16))
            self._keys.append(k)
        self._runs[key] = s0
        return s0


def _build_plan():
    maps = _build_maps()
    reg = _Slots()
    ident = np.eye(128)
    id_slot = reg.alloc_run([ident])

    # analysis: list of (name, src, runs); run = (s0, g, [(slot, bi0), ...])
    ana = []
    for name, src in (("d1", "xseg"), ("a1", "xseg"), ("d2", "a1"),
                      ("a2", "a1"), ("d3", "a2"), ("a3", "a2")):
        bl = _blockize(maps[name])
        runs = []
        s = 0
        while s < len(bl):
            ents = bl[s]
            g = 1
            while (g < 4 and s + g < len(bl) and len(bl[s + g]) == len(ents)
                   and all(b2 - b1 == 2 * g and np.array_equal(m2, m1)
                           for (b1, m1), (b2, m2) in zip(ents, bl[s + g]))):
                g += 1
            runs.append((s, g, [(reg.alloc_run([m]), bi) for bi, m in ents]))
            s += g
        ana.append((name, src, runs))

    # synthesis: per band: groups of 8 out-blocks (one [128,8,128] psum tile)
    # group = list of MMs (lhsT_bi, slot0, g, f) ; f = block offset in tile.
    # Runs never cross the 4-block bank boundary inside the tile.
    syn = []
    for name, src in (("b3", "d1"), ("b2", "d2"), ("b1", "d3"), ("b0", "a3")):
        bl = _blockize(maps[name])
        groups = []
        for g0 in range(0, len(bl), 8):
            mms = []
            s = g0
            while s < min(g0 + 8, len(bl)):
                f = s - g0
                cand = set(bi for bi, _ in bl[s])
                run = [s]
                cap = 4 - (f % 4)
                while (len(run) < cap and run[-1] + 1 < min(g0 + 8, len(bl))):
                    nxt = set(bi for bi, _ in bl[run[-1] + 1])
                    if cand & nxt:
                        cand = cand & nxt
                        run.append(run[-1] + 1)
                    else:
                        break
                main = max(cand, key=lambda bi: sum(
                    np.abs(dict(bl[s2])[bi]).sum() for s2 in run))
                slot0 = reg.alloc_run([dict(bl[s2])[main] for s2 in run])
                mms.append((main, slot0, len(run), f))
                for s2 in run:
                    for bi, m in bl[s2]:
                        if bi != main:
                            mms.append((bi, reg.alloc_run([m]), 1, s2 - g0))
                s = run[-1] + 1
            groups.append((g0, min(8, len(bl) - g0), mms))
        syn.append((name, src, groups))

    w_host = np.zeros((128, len(reg.mats), 128), np.float16)
    for j, m in enumerate(reg.mats):
        w_host[:, j, :] = m
    return {"ana": ana, "syn": syn, "id_slot": id_slot,
            "w_host": w_host.reshape(128, -1), "n_slots": len(reg.mats)}


_PLAN = None


def _get_plan():
    global _PLAN
    if _PLAN is None:
        _PLAN = _build_plan()
    return _PLAN


# ---------------------------------------------------------------------------
# numpy interpreter of the plan (host validation, mirrors emission exactly)
# ---------------------------------------------------------------------------

def _run_plan_numpy(x_rows, fp16=True):
    """x_rows [128, 8192] -> [4, 128, 8192] via the block plan."""
    plan = _get_plan()
    W = plan["w_host"].reshape(128, plan["n_slots"], 128).astype(np.float64)
    dt = np.float16 if fp16 else np.float64

    def wslot(s0, g):
        # [128, g*128] run view, as emitted (lhsT or rhs = M^T blocks)
        return W[:, s0:s0 + g, :].reshape(128, g * 128)

    xf = x_rows.astype(dt).astype(np.float64)
    seg = {}
    seg["xseg"] = np.stack([xf[:, b * 128:(b + 1) * 128].T for b in range(NB0)])
    seg["xseg"] = seg["xseg"].astype(dt).astype(np.float64)  # [nb, 128, 128]

    for name, src, runs in plan["ana"]:
        out = np.zeros((NB[name], 128, 128))
        for s0, g, ents in runs:
            ps = np.zeros((g, 128, 128))
            for slot, bi0 in ents:
                Wm = W[:, slot, :]          # M^T [pi, po]
                for j in range(g):
                    ps[j] += Wm.T @ seg[src][bi0 + 2 * j]
            out[s0:s0 + g] = ps
        seg[name] = out.astype(dt).astype(np.float64)

    bands = {}
    for name, src, groups in plan["syn"]:
        ob = np.zeros((128, L0))
        for g0, n, mms in groups:
            ps = np.zeros((n, 128, 128))   # [block, r, po]
            for bi, slot0, g, f in mms:
                blk = seg[src][bi]          # [pi, r]
                for j in range(g):
                    ps[f + j] += blk.T @ W[:, slot0 + j, :]
            for j in range(n):
                ob[:, (g0 + j) * 128:(g0 + j + 1) * 128] = ps[j]
        bands[name] = ob
    return np.stack([bands["b0"], bands["b1"], bands["b2"], bands["b3"]])


# ---------------------------------------------------------------------------
# bass emission
# ---------------------------------------------------------------------------

def build_nc():
    plan = _get_plan()
    ns = plan["n_slots"]
    nc = bacc.Bacc("TRN2", target_bir_lowering=False, debug=False,
                   num_devices=N_CORES)
    x_ap = nc.dram_tensor("x", [ROWS_PER_CORE, L0], F16, kind="ExternalInput").ap()
    w_ap = nc.dram_tensor("w", [128, ns * 128], F16, kind="ExternalInput").ap()
    y_ap = nc.dram_tensor("y", [4, ROWS_PER_CORE, L0], F16, kind="ExternalOutput").ap()

    with tile.TileContext(nc) as tc:
        with tc.tile_pool(name="w", bufs=1) as wpool, \
             tc.tile_pool(name="data", bufs=1) as pool, \
             tc.tile_pool(name="ob", bufs=3) as obpool, \
             tc.tile_pool(name="ps", bufs=4, space="PSUM") as pspool:

            wt = wpool.tile([128, ns, 128], F16, tag="w")
            wchunk = 16
            for c0 in range(0, ns, wchunk):
                n = min(wchunk, ns - c0)
                nc.sync.dma_start(wt[:, c0:c0 + n, :],
                                  w_ap[:, c0 * 128:(c0 + n) * 128])

            evac_tgl = [0]
            dma_tgl = [0]

            def store(dst_ap, src_ap):
                # alternate the two HWDGE rings (SP / ACT) for store issue
                if dma_tgl[0] == 0:
                    nc.sync.dma_start(dst_ap, src_ap)
                else:
                    nc.scalar.dma_start(dst_ap, src_ap)
                dma_tgl[0] ^= 1

            def evac(src_ap, dst_ap):
                # alternate PSUM->SBUF copies between DVE and ACT
                if evac_tgl[0] == 0:
                    nc.vector.tensor_copy(dst_ap, src_ap)
                else:
                    nc.scalar.copy(dst_ap, src_ap)
                evac_tgl[0] ^= 1

            for t in range(TILES_PER_CORE):
                rows = slice(t * 128, (t + 1) * 128)
                xf16 = pool.tile([128, NB0, 128], F16, tag="xf16")
                for c0 in range(0, NB0, 16):
                    nc.sync.dma_start(xf16[:, c0:c0 + 16, :],
                                      x_ap[rows, c0 * 128:(c0 + 16) * 128])

                xseg_t = pool.tile([128, NB0, 128], F16, tag="xseg", name="xseg")
                seg = {"xseg": xseg_t}
                for nm in ("d1", "a1", "d2", "a2", "d3", "a3"):
                    seg[nm] = pool.tile([128, NB[nm], 128], F16, tag=nm, name=nm)

                def emit_transpose():
                    for b0 in range(0, NB0, 8):
                        ps = pspool.tile([128, 8, 128], F32, tag="ps", name="ps")
                        for j in range(8):
                            nc.tensor.matmul(ps[:, j, :], xf16[:, b0 + j, :],
                                             wt[:, plan["id_slot"], :],
                                             start=(j % 4 == 0), stop=(j % 4 == 3))
                        evac(ps[:, :, :], seg["xseg"][:, b0:b0 + 8, :])

                def emit_ana(name, src, runs):
                    i = 0
                    while i < len(runs):
                        s0, g, ents = runs[i]
                        # pack up to 2 runs (banks) per psum tile
                        pack = [(s0, g, ents, 0)]
                        if (i + 1 < len(runs) and runs[i + 1][0] == s0 + g
                                and g <= 4 and runs[i + 1][1] <= 4):
                            s1, g1, e1 = runs[i + 1]
                            pack.append((s1, g1, e1, 4))
                            i += 1
                        i += 1
                        ps = pspool.tile([128, 8, 128], F32, tag="ps", name="ps")
                        for (rs, rg, re, f) in pack:
                            for j, (slot, bi0) in enumerate(re):
                                nc.tensor.matmul(
                                    ps[:, f:f + rg, :],
                                    wt[:, slot, :],
                                    seg[src][:, bi0:bi0 + 2 * (rg - 1) + 1:2, :],
                                    start=(j == 0), stop=(j == len(re) - 1))
                        if len(pack) == 2 and pack[1][0] == pack[0][0] + pack[0][1] \
                                and pack[0][1] == 4:
                            rs, rg = pack[0][0], pack[0][1] + pack[1][1]
                            evac(ps[:, 0:rg, :], seg[name][:, rs:rs + rg, :])
                        else:
                            for (rs, rg, re, f) in pack:
                                evac(ps[:, f:f + rg, :],
                                     seg[name][:, rs:rs + rg, :])

                def emit_syn(name, src, groups):
                    band = {"b0": 0, "b1": 1, "b2": 2, "b3": 3}[name]
                    ob = obpool.tile([128, L0], F16, tag="ob", name="ob")
                    half = len(groups) // 2
                    for gi, (g0, n, mms) in enumerate(groups):
                        ps = pspool.tile([128, 8, 128], F32, tag="ps", name="ps")
                        bank_first = {}
                        bank_last = {}
                        for k, (bi, slot0, g, f) in enumerate(mms):
                            bank = f // 4
                            bank_first.setdefault(bank, k)
                            bank_last[bank] = k
                        for k, (bi, slot0, g, f) in enumerate(mms):
                            bank = f // 4
                            nc.tensor.matmul(
                                ps[:, f:f + g, :],
                                seg[src][:, bi, :],
                                wt[:, slot0:slot0 + g, :],
                                start=(bank_first[bank] == k),
                                stop=(bank_last[bank] == k))
                        evac(ps[:, 0:n, :], ob[:, g0 * 128:(g0 + n) * 128])
                        # store each completed group (256KB) immediately
                        store(y_ap[band, rows, g0 * 128:(g0 + n) * 128],
                              ob[:, g0 * 128:(g0 + n) * 128])

                syn = {name: (name, src, groups)
                       for name, src, groups in plan["syn"]}
                ana = {name: (name, src, runs)
                       for name, src, runs in plan["ana"]}
                emit_transpose()
                emit_ana(*ana["d1"])
                emit_ana(*ana["a1"])
                emit_syn(*syn["b3"])
                emit_ana(*ana["d2"])
                emit_ana(*ana["a2"])
                emit_syn(*syn["b2"])
                emit_ana(*ana["d3"])
                emit_ana(*ana["a3"])
                emit_syn(*syn["b1"])
                emit_syn(*syn["b0"])

    nc.compile()
    return nc


_NC = None


def _get_nc():
    global _NC
    if _NC is None:
        _NC = build_nc()
    return _NC


def shard_inputs(x):
    plan = _get_plan()
    rows = np.ascontiguousarray(x.reshape(-1, L0).astype(np.float16))
    w = plan["w_host"]
    return [{"x": rows[c * ROWS_PER_CORE:(c + 1) * ROWS_PER_CORE], "w": w}
            for c in range(N_CORES)]


def unshard_outputs(results):
    out = np.empty((4, N_CORES * ROWS_PER_CORE, L0), np.float32)
    for c, r in enumerate(results):
        out[:, c * ROWS_PER_CORE:(c + 1) * ROWS_PER_CORE, :] = \
            r["y"].astype(np.float32)
    return out.reshape(4, 16, 128, L0)


def kernel(x):
    x = np.asarray(x, np.float32)
    assert x.shape == (16, 128, L0), x.shape
    nc = _get_nc()
    res = run_bass_kernel_spmd(nc, shard_inputs(x), core_ids=list(range(N_CORES)))
    return unshard_outputs(res.results)


# revision 5
# speedup vs baseline: 1.5230x; 1.5230x over previous
"""Trainium2 Bass kernel: 3-level db4 DWT front-end via PE banded matmuls.

Input  x : [16, 128, 8192] float32
Output   : [4, 16, 128, 8192] float32  (bands: approx, d3, d2, d1)

Strategy: 2048 independent rows sharded 256/core (2 tiles of 128 rows).
Per 128-row tile, all compute runs on the TensorEngine in a transposed
"segment" layout (signal positions on partitions, rows on the free axis):

  1. transpose-in: 64 matmuls  xseg_b = xrow_b^T @ I   (fp16)
  2. analysis: each level's dec_lo/dec_hi stride-2 conv (with reflect pad
     + odd-length handling folded into per-block constants) is a banded
     linear map; out block s = sum_j Msub_j @ in block (2s+d_j). Emitted as
     const-stationary matmuls: lhsT = Msub^T (shared across a run of up to
     4 output blocks), rhs = stride-2 block view of the input seg tile.
  3. synthesis: each band's full conv_transpose chain (1-3 stages, crop 7
     each) is composed host-side into one banded map from the deepest
     intermediate straight to the 8192-length band. Emitted data-stationary:
     lhsT = input seg block (data), rhs = per-output-block constant
     (Gsub^T), so the output lands directly in row layout [rows, positions]
     in PSUM - no transpose-out needed.

All constants are extracted at import time by applying the (numpy replica
of the) reference pipeline to identity matrices, so reflect padding, crop
offsets and tail truncation are exact by construction. fp16 data/weights
with fp32 PSUM accumulation gives rel err ~4e-4 (tolerance 2e-2).

PSUM rules honored: one matmul never crosses a 2KB bank; per bank the
first matmul carries start=True (lazy-zeroes the bank), the last stop=True.
"""

import numpy as np

import concourse.bass as bass
import concourse.tile as tile
from concourse import bacc, mybir
from concourse.bass_utils import run_bass_kernel_spmd

F32 = mybir.dt.float32
F16 = mybir.dt.float16

K = 8
DEC_LO = np.array([-0.0105974018, 0.0328830117, 0.0308413818, -0.1870348117,
                   -0.0279837694, 0.6308807679, 0.7148465706, 0.2303778133], np.float64)
DEC_HI = np.array([-0.2303778133, 0.7148465706, -0.6308807679, -0.0279837694,
                   0.1870348117, 0.0308413818, -0.0328830117, -0.0105974018], np.float64)
REC_LO = DEC_LO[::-1].copy()
REC_HI = DEC_HI[::-1].copy()

L0, L1, L2, L3 = 8192, 4100, 2054, 1031
N_CORES = 8
ROWS_PER_CORE = 256
TILES_PER_CORE = 2
NB0 = L0 // 128          # 64 x-blocks
NB = {"xseg": NB0, "d1": 33, "a1": 33, "d2": 17, "a2": 17, "d3": 9, "a3": 9}


# ---------------------------------------------------------------------------
# numpy replica of the reference pipeline (for map extraction)
# ---------------------------------------------------------------------------

def _dwconv_stride2(x, w):
    xp = np.pad(x, ((0, 0), (K - 1, K - 1)), mode="reflect")
    No = (xp.shape[1] - K) // 2 + 1
    out = np.zeros((x.shape[0], No), x.dtype)
    for k in range(K):
        out += w[k] * xp[:, k:k + 2 * No - 1:2]
    return out


def _dwconvT_stride2(x, w):
    N, L = x.shape
    up = np.zeros((N, 2 * L - 1), x.dtype)
    up[:, ::2] = x
    wf = w[::-1]
    upp = np.pad(up, ((0, 0), (K - 1, K - 1)))
    out = np.zeros((N, 2 * L + K - 2), x.dtype)
    for k in range(K):
        out += wf[k] * upp[:, k:k + out.shape[1]]
    return out


def _fit(out, target_len):
    Lc = out.shape[-1]
    if Lc > target_len:
        s = (Lc - target_len) // 2
        return out[..., s:s + target_len]
    if Lc < target_len:
        return np.pad(out, ((0, 0), (0, target_len - Lc)))
    return out


def _extract(op, in_len, out_pad):
    """Map matrix [out_pad, in_len] of linear `op` (zero-padded rows)."""
    M = op(np.eye(in_len)).T
    Mp = np.zeros((out_pad, in_len), np.float64)
    Mp[:M.shape[0]] = M
    return Mp


def _build_maps():
    P1, P2, P3 = NB["d1"] * 128, NB["d2"] * 128, NB["d3"] * 128
    mp = {}
    for nm, w_, li, op_ in (("d1", DEC_HI, L0, P1), ("a1", DEC_LO, L0, P1),
                            ("d2", DEC_HI, L1, P2), ("a2", DEC_LO, L1, P2),
                            ("d3", DEC_HI, L2, P3), ("a3", DEC_LO, L2, P3)):
        M = _extract(lambda I, w=w_: _dwconv_stride2(I, w), li, op_)
        # zero-pad input length to whole blocks (padded tail cols are zero)
        Mp = np.zeros((op_, -(-li // 128) * 128), np.float64)
        Mp[:, :li] = M
        mp[nm] = Mp

    def synth(chain, targets):
        def op(I):
            rec = I
            for w_, tgt in zip(chain, targets):
                rec = _fit(_dwconvT_stride2(rec, w_), tgt)
            return rec
        return op

    for nm, ch, tg, li in (("b3", [REC_HI], [L0], L1),
                           ("b2", [REC_HI, REC_LO], [L1, L0], L2),
                           ("b1", [REC_HI, REC_LO, REC_LO], [L2, L1, L0], L3),
                           ("b0", [REC_LO, REC_LO, REC_LO], [L2, L1, L0], L3)):
        M = _extract(synth(ch, tg), li, L0)
        Mp = np.zeros((L0, -(-li // 128) * 128), np.float64)
        Mp[:, :li] = M
        mp[nm] = Mp
    return mp


def _blockize(M):
    ob, ib = M.shape[0] // 128, M.shape[1] // 128
    out = []
    for s in range(ob):
        row = M[s * 128:(s + 1) * 128]
        ents = []
        for bi in range(ib):
            sub = row[:, bi * 128:(bi + 1) * 128]
            if np.abs(sub).max() > 0:
                ents.append((bi, sub))
        out.append(ents)
    return out


# ---------------------------------------------------------------------------
# plan construction
# ---------------------------------------------------------------------------

class _Slots:
    """fp16 constant [128,128] slot registry; contiguous runs dedup'd whole,
    including containment in previously stored slot sequences."""

    def __init__(self):
        self.mats = []          # list of np [128,128] float16 (stored as M^T)
        self._keys = []         # per-slot content hash
        self._runs = {}         # key(tuple of hashes) -> slot0

    def alloc_run(self, mats):
        key = tuple(np.asarray(m.T, np.float16).tobytes() for m in mats)
        if key in self._runs:
            return self._runs[key]
        n = len(key)
        for s0 in range(len(self._keys) - n + 1):
            if tuple(self._keys[s0:s0 + n]) == key:
                self._runs[key] = s0
                return s0
        s0 = len(self.mats)
        for m, k in zip(mats, key):
            self.mats.append(np.asarray(m.T, np.float16))
            self._keys.append(k)
        self._runs[key] = s0
        return s0


def _build_plan():
    maps = _build_maps()
    reg = _Slots()
    ident = np.eye(128)
    id_slot = reg.alloc_run([ident])

    # analysis: list of (name, src, runs); run = (s0, g, [(slot, bi0), ...])
    ana = []
    for name, src in (("d1", "xseg"), ("a1", "xseg"), ("d2", "a1"),
                      ("a2", "a1"), ("d3", "a2"), ("a3", "a2")):
        bl = _blockize(maps[name])
        runs = []
        s = 0
        while s < len(bl):
            ents = bl[s]
            g = 1
            while (g < 4 and s + g < len(bl) and len(bl[s + g]) == len(ents)
                   and all(b2 - b1 == 2 * g and np.array_equal(m2, m1)
                           for (b1, m1), (b2, m2) in zip(ents, bl[s + g]))):
                g += 1
            runs.append((s, g, [(reg.alloc_run([m]), bi) for bi, m in ents]))
            s += g
        ana.append((name, src, runs))

    # synthesis: per band: groups of 8 out-blocks (one [128,8,128] psum tile)
    # group = list of MMs (lhsT_bi, slot0, g, f) ; f = block offset in tile.
    # Runs never cross the 4-block bank boundary inside the tile.
    syn = []
    for name, src in (("b3", "d1"), ("b2", "d2"), ("b1", "d3"), ("b0", "a3")):
        bl = _blockize(maps[name])
        groups = []
        for g0 in range(0, len(bl), 8):
            mms = []
            s = g0
            while s < min(g0 + 8, len(bl)):
                f = s - g0
                cand = set(bi for bi, _ in bl[s])
                run = [s]
                cap = 4 - (f % 4)
                while (len(run) < cap and run[-1] + 1 < min(g0 + 8, len(bl))):
                    nxt = set(bi for bi, _ in bl[run[-1] + 1])
                    if cand & nxt:
                        cand = cand & nxt
                        run.append(run[-1] + 1)
                    else:
                        break
                main = max(cand, key=lambda bi: sum(
                    np.abs(dict(bl[s2])[bi]).sum() for s2 in run))
                slot0 = reg.alloc_run([dict(bl[s2])[main] for s2 in run])
                mms.append((main, slot0, len(run), f))
                for s2 in run:
                    for bi, m in bl[s2]:
                        if bi != main:
                            mms.append((bi, reg.alloc_run([m]), 1, s2 - g0))
                s = run[-1] + 1
            groups.append((g0, min(8, len(bl) - g0), mms))
        syn.append((name, src, groups))

    w_host = np.zeros((128, len(reg.mats), 128), np.float16)
    for j, m in enumerate(reg.mats):
        w_host[:, j, :] = m
    return {"ana": ana, "syn": syn, "id_slot": id_slot,
            "w_host": w_host.reshape(128, -1), "n_slots": len(reg.mats)}


_PLAN = None


def _get_plan():
    global _PLAN
    if _PLAN is None:
        _PLAN = _build_plan()
    return _PLAN


# ---------------------------------------------------------------------------
# numpy interpreter of the plan (host validation, mirrors emission exactly)
# ---------------------------------------------------------------------------

def _run_plan_numpy(x_rows, fp16=True):
    """x_rows [128, 8192] -> [4, 128, 8192] via the block plan."""
    plan = _get_plan()
    W = plan["w_host"].reshape(128, plan["n_slots"], 128).astype(np.float64)
    dt = np.float16 if fp16 else np.float64

    def wslot(s0, g):
        # [128, g*128] run view, as emitted (lhsT or rhs = M^T blocks)
        return W[:, s0:s0 + g, :].reshape(128, g * 128)

    xf = x_rows.astype(dt).astype(np.float64)
    seg = {}
    seg["xseg"] = np.stack([xf[:, b * 128:(b + 1) * 128].T for b in range(NB0)])
    seg["xseg"] = seg["xseg"].astype(dt).astype(np.float64)  # [nb, 128, 128]

    for name, src, runs in plan["ana"]:
        out = np.zeros((NB[name], 128, 128))
        for s0, g, ents in runs:
            ps = np.zeros((g, 128, 128))
            for slot, bi0 in ents:
                Wm = W[:, slot, :]          # M^T [pi, po]
                for j in range(g):
                    ps[j] += Wm.T @ seg[src][bi0 + 2 * j]
            out[s0:s0 + g] = ps
        seg[name] = out.astype(dt).astype(np.float64)

    bands = {}
    for name, src, groups in plan["syn"]:
        ob = np.zeros((128, L0))
        for g0, n, mms in groups:
            ps = np.zeros((n, 128, 128))   # [block, r, po]
            for bi, slot0, g, f in mms:
                blk = seg[src][bi]          # [pi, r]
                for j in range(g):
                    ps[f + j] += blk.T @ W[:, slot0 + j, :]
            for j in range(n):
                ob[:, (g0 + j) * 128:(g0 + j + 1) * 128] = ps[j]
        bands[name] = ob
    return np.stack([bands["b0"], bands["b1"], bands["b2"], bands["b3"]])


# ---------------------------------------------------------------------------
# bass emission
# ---------------------------------------------------------------------------

def build_nc():
    plan = _get_plan()
    ns = plan["n_slots"]
    nc = bacc.Bacc("TRN2", target_bir_lowering=False, debug=False,
                   num_devices=N_CORES)
    x_ap = nc.dram_tensor("x", [ROWS_PER_CORE, L0], F16, kind="ExternalInput").ap()
    w_ap = nc.dram_tensor("w", [128, ns * 128], F16, kind="ExternalInput").ap()
    y_ap = nc.dram_tensor("y", [4, ROWS_PER_CORE, L0], F16, kind="ExternalOutput").ap()

    with tile.TileContext(nc) as tc:
        with tc.tile_pool(name="w", bufs=1) as wpool, \
             tc.tile_pool(name="data", bufs=1) as pool, \
             tc.tile_pool(name="ob", bufs=3) as obpool, \
             tc.tile_pool(name="ps", bufs=4, space="PSUM") as pspool:

            wt = wpool.tile([128, ns, 128], F16, tag="w")
            nc.sync.dma_start(wt[:, :, :], w_ap[:, :])

            evac_tgl = [0]

            def evac(src_ap, dst_ap):
                # alternate PSUM->SBUF copies between DVE and ACT
                if evac_tgl[0] == 0:
                    nc.vector.tensor_copy(dst_ap, src_ap)
                else:
                    nc.scalar.copy(dst_ap, src_ap)
                evac_tgl[0] ^= 1

            for t in range(TILES_PER_CORE):
                rows = slice(t * 128, (t + 1) * 128)
                xf16 = pool.tile([128, NB0, 128], F16, tag="xf16")
                for c0 in range(0, NB0, 16):
                    nc.sync.dma_start(xf16[:, c0:c0 + 16, :],
                                      x_ap[rows, c0 * 128:(c0 + 16) * 128])

                xseg_t = pool.tile([128, NB0, 128], F16, tag="xseg", name="xseg")
                seg = {"xseg": xseg_t}
                for nm in ("d1", "a1", "d2", "a2", "d3", "a3"):
                    seg[nm] = pool.tile([128, NB[nm], 128], F16, tag=nm, name=nm)

                def emit_transpose():
                    for b0 in range(0, NB0, 8):
                        ps = pspool.tile([128, 8, 128], F32, tag="ps", name="ps")
                        for j in range(8):
                            nc.tensor.matmul(ps[:, j, :], xf16[:, b0 + j, :],
                                             wt[:, plan["id_slot"], :],
                                             start=(j % 4 == 0), stop=(j % 4 == 3))
                        evac(ps[:, :, :], seg["xseg"][:, b0:b0 + 8, :])

                def emit_ana(name, src, runs):
                    i = 0
                    while i < len(runs):
                        s0, g, ents = runs[i]
                        # pack up to 2 runs (banks) per psum tile
                        pack = [(s0, g, ents, 0)]
                        if (i + 1 < len(runs) and runs[i + 1][0] == s0 + g
                                and g <= 4 and runs[i + 1][1] <= 4):
                            s1, g1, e1 = runs[i + 1]
                            pack.append((s1, g1, e1, 4))
                            i += 1
                        i += 1
                        ps = pspool.tile([128, 8, 128], F32, tag="ps", name="ps")
                        for (rs, rg, re, f) in pack:
                            for j, (slot, bi0) in enumerate(re):
                                nc.tensor.matmul(
                                    ps[:, f:f + rg, :],
                                    wt[:, slot, :],
                                    seg[src][:, bi0:bi0 + 2 * (rg - 1) + 1:2, :],
                                    start=(j == 0), stop=(j == len(re) - 1))
                        if len(pack) == 2 and pack[1][0] == pack[0][0] + pack[0][1] \
                                and pack[0][1] == 4:
                            rs, rg = pack[0][0], pack[0][1] + pack[1][1]
                            evac(ps[:, 0:rg, :], seg[name][:, rs:rs + rg, :])
                        else:
                            for (rs, rg, re, f) in pack:
                                evac(ps[:, f:f + rg, :],
                                     seg[name][:, rs:rs + rg, :])

                def emit_syn(name, src, groups):
                    band = {"b0": 0, "b1": 1, "b2": 2, "b3": 3}[name]
                    ob = obpool.tile([128, L0], F16, tag="ob", name="ob")
                    half = len(groups) // 2
                    for gi, (g0, n, mms) in enumerate(groups):
                        ps = pspool.tile([128, 8, 128], F32, tag="ps", name="ps")
                        bank_first = {}
                        bank_last = {}
                        for k, (bi, slot0, g, f) in enumerate(mms):
                            bank = f // 4
                            bank_first.setdefault(bank, k)
                            bank_last[bank] = k
                        for k, (bi, slot0, g, f) in enumerate(mms):
                            bank = f // 4
                            nc.tensor.matmul(
                                ps[:, f:f + g, :],
                                seg[src][:, bi, :],
                                wt[:, slot0:slot0 + g, :],
                                start=(bank_first[bank] == k),
                                stop=(bank_last[bank] == k))
                        evac(ps[:, 0:n, :], ob[:, g0 * 128:(g0 + n) * 128])
                        # store each completed group (256KB) immediately
                        nc.sync.dma_start(y_ap[band, rows, g0 * 128:(g0 + n) * 128],
                                          ob[:, g0 * 128:(g0 + n) * 128])

                syn = {name: (name, src, groups)
                       for name, src, groups in plan["syn"]}
                ana = {name: (name, src, runs)
                       for name, src, runs in plan["ana"]}
                emit_transpose()
                emit_ana(*ana["d1"])
                emit_ana(*ana["a1"])
                emit_syn(*syn["b3"])
                emit_ana(*ana["d2"])
                emit_ana(*ana["a2"])
                emit_syn(*syn["b2"])
                emit_ana(*ana["d3"])
                emit_ana(*ana["a3"])
                emit_syn(*syn["b1"])
                emit_syn(*syn["b0"])

    nc.compile()
    return nc


_NC = None


def _get_nc():
    global _NC
    if _NC is None:
        _NC = build_nc()
    return _NC


def shard_inputs(x):
    plan = _get_plan()
    rows = np.ascontiguousarray(x.reshape(-1, L0).astype(np.float16))
    w = plan["w_host"]
    return [{"x": rows[c * ROWS_PER_CORE:(c + 1) * ROWS_PER_CORE], "w": w}
            for c in range(N_CORES)]


def unshard_outputs(results):
    out = np.empty((4, N_CORES * ROWS_PER_CORE, L0), np.float32)
    for c, r in enumerate(results):
        out[:, c * ROWS_PER_CORE:(c + 1) * ROWS_PER_CORE, :] = \
            r["y"].astype(np.float32)
    return out.reshape(4, 16, 128, L0)


def kernel(x):
    x = np.asarray(x, np.float32)
    assert x.shape == (16, 128, L0), x.shape
    nc = _get_nc()
    res = run_bass_kernel_spmd(nc, shard_inputs(x), core_ids=list(range(N_CORES)))
    return unshard_outputs(res.results)
